# revision 32
# baseline (speedup 1.0000x reference)
"""Trainium2 Bass kernel for the 3-layer sparse-attention model (nn_BDH).

Self-contained: kernel(**inputs) takes the FULL inputs (as produced by
setup_inputs()) and returns the FULL [B, T, OUT] logits, distributing work
over 8 NeuronCores as (batch x head-pair): core c handles batch c//4 and
heads {2*(c%4), 2*(c%4)+1}. Per layer, the per-head decoder partials are
summed with a grouped AllReduce within each batch's 4-core group.

Pipelined schedule: each layer runs in 8 chunks of 256 tokens, chunk-major
over the two heads, so a chunk's yMLP partial finishes early and its (bf16)
AllReduce overlaps the remaining chunks' compute. The post-AR LayerNorm
chain + transposes for chunk c of layer l+1 are emitted FOUR chunk-groups
early (inside layer l's stream), so their vector-engine latency hides under
attention matmuls and the tensor engine never waits on them at layer
boundaries. xs is double-buffered across layers so the next layer's LN
writes don't collide with this layer's attention reads. The two heads'
score/yKV stages are emitted back-to-back before either head's stats/Zy
tail, giving the tensor engine a long dependency-free runway.

Numerics: bf16 operands everywhere except the input projection, the
residual-stream xs tiles, score tiles and the LN chains (f32/f32r);
validated host-side and on HW at ~7e-3 scale-relative absmax error vs the
fp32 reference (tolerance 2e-2). bf16 matmuls run at full PE rate at any
free-dim size.

Layout/algebra tricks (from the validated baseline):
- The n-axis of encoder/encoder_v/decoder is de-interleaved host-side, so
  RoPE becomes a half-split rotation with [128, T] cos/sin tables.
- Scores are symmetric (S = QR @ QR^T); the strict-causal mask becomes a
  strict-UPPER mask on S in [s, t] layout, which is exactly the lhsT the
  yKV matmul wants -- the attention inner loop is transpose-free.
- yKV is produced transposed ([d, t]); its LayerNorm is folded away: the
  mean correction enters the encoder_v matmul as a K=1 rank-1 update with
  host-precomputed -sum_d(encoder_v), and 1/sigma is folded into the
  decoder-output accumulation as a per-partition scalar.
"""
import math
import os

import numpy as np

_BUILT = None
LAST_RESULTS = None  # BassKernelResults of the most recent run (for test.py)

B, T, D, NH, MULT, NL = 2, 2048, 512, 8, 4, 3
N = D * MULT // NH          # 256 per-head latent dim
NHALF = N // 2
OUT = 512
EPS = 1e-5
DC = D // 128               # 4 d-chunks
TB = T // 128               # 16 t-blocks
CHUNK = 256                 # attention chunk width
NCH = T // CHUNK            # 8 chunks per layer
TH = T // 2                 # half length for [128, TH] state tiles
LAG = 4                     # ln-chunk emission lead (in chunk groups)


def round_fp32r(x):
    b = np.ascontiguousarray(x, dtype=np.float32).view(np.uint32).astype(np.uint64)
    b = b + 0x7FF + ((b >> np.uint64(12)) & np.uint64(1))
    return (b & np.uint64(0xFFFFF000)).astype(np.uint32).view(np.float32)


def build():
    from contextlib import ExitStack

    import concourse.bacc as bacc
    import concourse.tile as tile
    import concourse.tile_utils as tile_utils
    from concourse import mybir

    if getattr(tile_utils, "max_sbuf_usage", 0) < 208 * 1024:
        tile_utils.max_sbuf_usage = 208 * 1024

    f32 = mybir.dt.float32
    f32r = mybir.dt.float32r
    bf16 = mybir.dt.bfloat16
    AF = mybir.ActivationFunctionType
    ALU = mybir.AluOpType

    nc = bacc.Bacc("TRN2", target_bir_lowering=False, debug=False, num_devices=8)

    xT_p = nc.declare_dram_parameter("xT", [DC, 128, T], f32r, isOutput=False)
    w_in_p = nc.declare_dram_parameter("w_in", [DC, 128, D], f32r, isOutput=False)
    b_in_p = nc.declare_dram_parameter("b_in_row", [1, D], f32r, isOutput=False)
    enc_p = nc.declare_dram_parameter("enc", [2, DC, 128, N], bf16, isOutput=False)
    encv_p = nc.declare_dram_parameter("encv", [2, DC, 128, N], bf16,
                                       isOutput=False)
    dec_p = nc.declare_dram_parameter("dec", [2, 2, NHALF, D], bf16,
                                      isOutput=False)
    cos_p = nc.declare_dram_parameter("cosT", [NHALF, T], bf16, isOutput=False)
    sin_p = nc.declare_dram_parameter("sinT", [NHALF, T], bf16, isOutput=False)
    maskP_p = nc.declare_dram_parameter("maskP", [128, 4 * 128], bf16,
                                        isOutput=False)
    ident_p = nc.declare_dram_parameter("ident", [128, 128], bf16, isOutput=False)
    onesd_p = nc.declare_dram_parameter("onesd", [128, 1], bf16, isOutput=False)
    ones128_p = nc.declare_dram_parameter("ones128", [1, 128], f32r,
                                          isOutput=False)
    hw_p = nc.declare_dram_parameter("head_w", [DC, 128, OUT], bf16,
                                     isOutput=False)
    hb_p = nc.declare_dram_parameter("head_b_col", [128, OUT // 128], f32,
                                     isOutput=False)
    out_p = nc.declare_dram_parameter("logitsT", [OUT, T], f32, isOutput=True)

    with tile.TileContext(nc) as tc, ExitStack() as ctx:
        const = ctx.enter_context(tc.tile_pool(name="const", bufs=1))
        state = ctx.enter_context(tc.tile_pool(name="state", bufs=1))
        wstream = ctx.enter_context(tc.tile_pool(name="wstream", bufs=1))
        work = ctx.enter_context(tc.tile_pool(name="work", bufs=1))
        spool = ctx.enter_context(tc.tile_pool(name="spool", bufs=1))
        small = ctx.enter_context(tc.tile_pool(name="small", bufs=4))
        psum = ctx.enter_context(tc.tile_pool(name="psum", bufs=1, space="PSUM"))
        dram = ctx.enter_context(tc.tile_pool(name="dram", bufs=1, space="DRAM"))

        # ---------------- constants ----------------
        cosT = const.tile([NHALF, T], bf16)
        sinT = const.tile([NHALF, T], bf16)
        maskP = const.tile([128, 4 * 128], bf16)
        ident = const.tile([128, 128], bf16)
        onesd = const.tile([128, 1], bf16)
        ones128 = const.tile([1, 128], f32r)

        b_in_row = const.tile([1, D], f32r)
        hb_col = const.tile([128, OUT // 128], f32)
        epsc = const.tile([128, 1], f32)
        nc.sync.dma_start(ident[:], ident_p[:])
        nc.sync.dma_start(ones128[:], ones128_p[:])
        nc.sync.dma_start(b_in_row[:], b_in_p[:])
        nc.vector.memset(epsc[:], EPS)

        # ------------- persistent state -------------
        xs2 = [[state.tile([128, D], bf16, name=f"xs{p}_{tb}")
                for tb in range(TB)] for p in range(2)]
        xsT = [[state.tile([128, TH], bf16, name=f"xsT{dc}_{hf}")
                for hf in range(2)] for dc in range(DC)]
        QR = [[[[state.tile([128, TH], bf16, name=f"QR{p}_{h}_{nt}_{hf}")
                 for hf in range(2)] for nt in range(2)] for h in range(2)]
              for p in range(2)]
        msq_col = [state.tile([128, TB], bf16, name=f"msq_col{h}")
                   for h in range(2)]
        rsig_col = [state.tile([128, TB], f32, name=f"rsig_col{h}")
                    for h in range(2)]

        def half_ap(tiles2, c0, c1):
            """AP for columns [c0:c1) of a T-range stored as two TH tiles.
            The range must not cross the half boundary."""
            hf = c0 // TH
            assert (c1 - 1) // TH == hf, (c0, c1)
            return tiles2[hf][:, c0 - hf * TH:c1 - hf * TH]

        ar_in = [dram.tile([CHUNK, D], bf16, name=f"ar_in{c}")
                 for c in range(NCH)]
        ar_out = [dram.tile([CHUNK, D], bf16, name=f"ar_out{c}")
                  for c in range(NCH)]
        msq_b = dram.tile([1, CHUNK], bf16, name="msq_b", tag="msq_b", bufs=2)

        def ln_tile(dst_ap, src_ap):
            bn6 = small.tile([128, 6], f32, name="bn6", tag="bn6")
            bn2 = small.tile([128, 2], f32, name="bn2", tag="bn2")
            sd = small.tile([128, 1], f32, name="sd", tag="sd")
            rs = small.tile([128, 1], f32, name="rs", tag="rs")
            nc.vector.bn_stats(bn6[:], src_ap)
            nc.vector.bn_aggr(bn2[:], bn6[:])
            nc.scalar.activation(sd[:], bn2[:, 1:2], AF.Sqrt, bias=epsc[:])
            nc.vector.reciprocal(rs[:], sd[:])
            nc.vector.tensor_scalar(dst_ap, src_ap, bn2[:, 0:1], rs[:],
                                    ALU.subtract, ALU.mult)

        def transpose_block(xs_tile, tb):
            """xsT[:, tb-block] = xs_tile^T (per d-chunk)."""
            hf, tbl = divmod(tb, TB // 2)
            for dc in range(DC):
                pt = psum.tile([128, 128], bf16, name="ptr", tag="ykv", bufs=2)
                nc.tensor.transpose(pt[:], xs_tile[:, dc * 128:(dc + 1) * 128],
                                    ident[:])
                dst = xsT[dc][hf][:, tbl * 128:(tbl + 1) * 128]
                if (dc + tbl) % 2 == 0:
                    nc.vector.tensor_copy(dst, pt[:])
                else:
                    nc.scalar.activation(dst, pt[:], AF.Copy)

        # =========================================================
        # stage A: input projection / post-AR LN per chunk
        # =========================================================
        w_in_sb = [wstream.tile([128, D], f32r, name=f"win{dc}",
                                tag=f"w{dc}") for dc in range(DC)]
        for dc in range(DC):
            nc.sync.dma_start(w_in_sb[dc][:], w_in_p[dc])
        xin = [None]
        nc.sync.dma_start(cosT[:], cos_p[:])
        nc.sync.dma_start(sinT[:], sin_p[:])
        nc.sync.dma_start(maskP[:], maskP_p[:])
        nc.sync.dma_start(onesd[:], onesd_p[:])
        nc.sync.dma_start(hb_col[:], hb_p[:])

        def input_proj_chunk(c):
            """xs_0 chunk c = ln(x @ w_in + b_in), plus transposes."""
            if c % 2 == 0:
                xin[0] = [wstream.tile([128, 2 * CHUNK], f32r,
                                       name=f"xin{dc}_{c}", tag=f"xin{dc}")
                          for dc in range(DC)]
                for dc in range(DC):
                    nc.sync.dma_start(
                        xin[0][dc][:],
                        xT_p[dc, :, c * CHUNK:(c + 2) * CHUNK])
            gc0 = (c // 2) * 2 * CHUNK
            for bi in range(CHUNK // 128):
                tb = (CHUNK // 128) * c + bi
                pz = psum.tile([128, D], f32, name="pz", tag="ym", bufs=2)
                for dc in range(DC):
                    nc.tensor.matmul(
                        pz[:],
                        xin[0][dc][:, tb * 128 - gc0:tb * 128 - gc0 + 128],
                        w_in_sb[dc][:], start=(dc == 0), stop=False)
                nc.tensor.matmul(pz[:], ones128[:], b_in_row[:], start=False,
                                 stop=True)
                ln_tile(xs2[0][tb][:], pz[:])
                transpose_block(xs2[0][tb], tb)

        def ln_chunk(lyr, c):
            """xs_{lyr} chunk c = ln(xs_{lyr-1} + ln(AR(yMLP_{lyr-1}))).
            Reads ar_out[c] (bf16), residual xs_{lyr-1}; writes xs2[lyr % 2]
            and xsT chunk c."""
            src = xs2[(lyr - 1) % 2]
            dst = xs2[lyr % 2]
            for bi in range(CHUNK // 128):
                tb = (CHUNK // 128) * c + bi
                yt = work.tile([128, D], bf16, name="ln_in", tag="ln_in",
                               bufs=2)
                nc.sync.dma_start(yt[:],
                                  ar_out[c][bi * 128:(bi + 1) * 128, :])
                n1 = work.tile([128, D], f32, name="ln_n1", tag="ln_n1",
                               bufs=2)
                ln_tile(n1[:], yt[:])
                u = work.tile([128, D], f32, name="ln_u", tag="ln_u")
                nc.gpsimd.tensor_tensor(u[:], n1[:], src[tb][:], ALU.add)
                ln_tile(dst[tb][:], u[:])
                transpose_block(dst[tb], tb)

        # =========================================================
        # per-layer per-chunk compute
        # =========================================================
        enc_sb = [[wstream.tile([128, N], bf16, name=f"enc{h}{dc}",
                                tag=f"e{h}{dc}", bufs=2) for dc in range(DC)]
                  for h in range(2)]
        encv_sb = [[wstream.tile([128, N], bf16, name=f"env{h}{dc}",
                                 tag=f"v{h}{dc}", bufs=2) for dc in range(DC)]
                   for h in range(2)]
        dec_sb = [[wstream.tile([NHALF, D], bf16, name=f"dec{h}{nt}",
                                tag=f"d{h}{nt}", bufs=2) for nt in range(2)]
                  for h in range(2)]

        hw_sb = [wstream.tile([128, OUT], bf16, name=f"hw{dc}", tag=f"w{dc}")
                 for dc in range(DC)]

        def load_weights(lyr):
            for h in range(2):
                for dc in range(DC):
                    nc.sync.dma_start(enc_sb[h][dc][:], enc_p[h, dc])
                for dc in range(DC):
                    nc.sync.dma_start(encv_sb[h][dc][:], encv_p[h, dc])
                for nt in range(2):
                    nc.sync.dma_start(dec_sb[h][nt][:], dec_p[h, nt])

        def zq_rope_pair(lyr, h, c):
            """Q/QR for chunks c, c+1 (512 wide) for head h (bf16).
            Returns the transient Q tile [nt] (512 cols)."""
            QRl = QR[lyr % 2][h]
            c0, c1 = c * CHUNK, (c + 2) * CHUNK
            W = c1 - c0
            zq = []
            for nt in range(2):
                pq = psum.tile([128, W], f32, name=f"zq{nt}", tag="sz",
                               bufs=4)
                for dc in range(DC):
                    nc.tensor.matmul(
                        pq[:], enc_sb[h][dc][:, nt * 128:(nt + 1) * 128],
                        half_ap(xsT[dc], c0, c1),
                        start=(dc == 0), stop=(dc == DC - 1))
                zq.append(pq)
            Qc = [work.tile([128, W], bf16, name=f"Qc{nt}",
                            tag=f"Qc{nt}", bufs=6) for nt in range(2)]
            for nt in range(2):
                nc.scalar.activation(Qc[nt][:], zq[nt][:], AF.Relu)
            ta = work.tile([128, W], bf16, name="ropeA", tag="rtA")
            tb_ = work.tile([128, W], bf16, name="ropeB", tag="rtB")
            nc.vector.tensor_tensor(ta[:], Qc[0][:], cosT[:, c0:c1], ALU.mult)
            nc.vector.tensor_tensor(tb_[:], Qc[1][:], sinT[:, c0:c1],
                                    ALU.mult)
            nc.vector.tensor_tensor(half_ap(QRl[0], c0, c1), ta[:], tb_[:],
                                    ALU.subtract)
            ta2 = work.tile([128, W], bf16, name="ropeA2", tag="rtA")
            tb2 = work.tile([128, W], bf16, name="ropeB2", tag="rtB")
            nc.vector.tensor_tensor(ta2[:], Qc[1][:], cosT[:, c0:c1],
                                    ALU.mult)
            nc.vector.tensor_tensor(tb2[:], Qc[0][:], sinT[:, c0:c1],
                                    ALU.mult)
            nc.vector.tensor_tensor(half_ap(QRl[1], c0, c1), ta2[:],
                                    tb2[:], ALU.add)
            return Qc

        def scores_ykv(lyr, h, c):
            """Masked scores + yKV^T + squares for head h, chunk c.
            Score j-blocks are computed in PAIRS into one 512-wide psum
            tile and evacuated with a single op. The top diagonal block
            (s0 = t0+128) is zero in its first 128 t-columns, so only its
            last 128 columns are computed (psum cols 384:512; cols 256:384
            are zeroed by the mask on evacuation) and the yKV contraction
            reads only that half. yKV d-tiles are likewise paired into
            512-wide psum tiles with a single evacuation + square each.
            Returns ykv2_sb: [2] tiles of [128, 512] (d-tile dp covers
            d-chunks 2dp, 2dp+1) and sq2_sb likewise."""
            QRl = QR[lyr % 2][h]
            xs_cur = xs2[lyr % 2]
            t0, t1 = c * CHUNK, (c + 1) * CHUNK
            nsb = t1 // 128
            npair = nsb // 2
            s2_tiles = []
            for p in range(npair):
                last = p == npair - 1
                ps = psum.tile([128, 2 * CHUNK], f32, name="ps_s", tag="sz",
                               bufs=4)
                for half in range(2):
                    j = 2 * p + half
                    s0 = j * 128
                    if last and half == 1:
                        lo, wid = 384, 128
                    else:
                        lo, wid = half * CHUNK, CHUNK
                    for nt in range(2):
                        nc.tensor.matmul(ps[:, lo:lo + wid],
                                         half_ap(QRl[nt], s0, s0 + 128),
                                         half_ap(QRl[nt], t1 - wid, t1)
                                         if last and half == 1 else
                                         half_ap(QRl[nt], t0, t1),
                                         start=(nt == 0), stop=(nt == 1),
                                         skip_group_check=True)
                st = spool.tile([128, 2 * CHUNK], bf16, name=f"s{p}",
                                tag=f"s{p}")
                if last:
                    nc.vector.tensor_tensor(st[:], ps[:], maskP[:], ALU.mult)
                elif (p % 2 == 0) if c >= 5 else (p % 3 == 0):
                    nc.vector.tensor_copy(st[:], ps[:])
                else:
                    nc.scalar.activation(st[:], ps[:], AF.Copy)
                s2_tiles.append(st)

            ykv2_sb = [work.tile([128, 2 * CHUNK], bf16, name=f"ykvsb{dp}",
                                 tag=f"ykvsb{dp}", bufs=4) for dp in range(2)]
            sq2_sb = [work.tile([128, 2 * CHUNK], bf16, name=f"sqsb{dp}",
                                tag=f"sqsb{dp}", bufs=4) for dp in range(2)]
            for dp in range(2):
                pykv = psum.tile([128, 2 * CHUNK], f32, name="pykv",
                                 tag="ykv", bufs=2)
                for half in range(2):
                    dt = 2 * dp + half
                    off = half * CHUNK
                    for j in range(nsb - 1):
                        nc.tensor.matmul(
                            pykv[:, off:off + CHUNK],
                            xs_cur[j][:, dt * 128:(dt + 1) * 128],
                            s2_tiles[j // 2][:, (j % 2) * CHUNK:
                                             (j % 2 + 1) * CHUNK],
                            start=(j == 0), stop=False,
                            skip_group_check=True)
                    nc.tensor.matmul(
                        pykv[:, off + 128:off + CHUNK],
                        xs_cur[nsb - 1][:, dt * 128:(dt + 1) * 128],
                        s2_tiles[npair - 1][:, 384:512],
                        start=False, stop=(half == 1),
                        skip_group_check=True)
                nc.scalar.activation(ykv2_sb[dp][:], pykv[:], AF.Copy)
                nc.gpsimd.tensor_tensor(sq2_sb[dp][:], ykv2_sb[dp][:],
                                        ykv2_sb[dp][:], ALU.mult)
            return ykv2_sb, sq2_sb

        def stats_rsig(h, c, sq_sb):
            """rsigma of yKV over d (yKV is zero-mean by construction:
            xs rows are LN outputs, so E_d[yKV] ~ 0 and the variance is
            just E_d[yKV^2]). Emitted ahead of the Zy/yMLP tail so the
            psum->row->DRAM-bounce->col->rsqrt chain hides under it."""
            pmsq = psum.tile([1, CHUNK], f32, name="pmsq", tag="ykv", bufs=2)
            for dt in range(DC):
                nc.tensor.matmul(
                    pmsq[:], onesd[:],
                    sq_sb[dt // 2][:, (dt % 2) * CHUNK:(dt % 2 + 1) * CHUNK],
                    start=(dt == 0), stop=(dt == DC - 1))
            msq_row = work.tile([1, CHUNK], bf16, name="msq_row",
                                tag="msq_row", bufs=2)
            nc.vector.tensor_copy(msq_row[:], pmsq[:])
            nc.sync.dma_start(msq_b[:], msq_row[:])
            nc.sync.dma_start(
                msq_col[h][:, 2 * c:2 * c + 2],
                msq_b[:].rearrange("one (c p) -> (one p) c", p=128))
            c2 = slice(2 * c, 2 * c + 2)
            tsd = small.tile([128, 2], f32, name="tsd", tag="tsd")
            nc.scalar.activation(tsd[:], msq_col[h][:, c2], AF.Sqrt,
                                 bias=epsc[:])
            nc.vector.reciprocal(rsig_col[h][:, c2], tsd[:])

        def attn_tail(lyr, h, c, Qc, ykv_sb, sq_sb, yacc):
            """stats, Zy, xy, yMLP for head h chunk c. h=0 writes yacc
            (bf16), h=1 accumulates into yar (bf16) and returns it."""
            t0 = c * CHUNK
            qoff = (c % 2) * CHUNK
            # ---- Zy -> xy ----
            xy_sb = [work.tile([128, CHUNK], bf16, name=f"xy{nt}",
                               tag=f"xy{nt}", bufs=2) for nt in range(2)]
            for nt in range(2):
                pzy = psum.tile([128, CHUNK], f32, name="pzy", tag="sz",
                                bufs=4)
                for dc in range(DC):
                    nc.tensor.matmul(
                        pzy[:], encv_sb[h][dc][:, nt * 128:(nt + 1) * 128],
                        ykv_sb[dc // 2][:, (dc % 2) * CHUNK:
                                        (dc % 2 + 1) * CHUNK],
                        start=(dc == 0), stop=(dc == DC - 1))
                nc.vector.scalar_tensor_tensor(
                    xy_sb[nt][:], pzy[:], 0.0,
                    Qc[nt][:, qoff:qoff + CHUNK], ALU.max, ALU.mult)

            # ---- yMLP partial (rsig folded into evacuation) ----
            out_tiles = []
            for bi in range(CHUNK // 128):
                tb = 2 * c + bi
                pym = psum.tile([128, D], f32, name="pym", tag="ym", bufs=2)
                for nt in range(2):
                    nc.tensor.matmul(
                        pym[:], xy_sb[nt][:, bi * 128:(bi + 1) * 128],
                        dec_sb[h][nt][:], start=(nt == 0), stop=(nt == 1))
                # plain evacuation first: frees the psum slot without
                # waiting on the rsig chain; scale applied from SBUF
                ymt = work.tile([128, D], bf16, name=f"ymt{bi}",
                                tag=f"ymt{bi}", bufs=2)
                nc.scalar.activation(ymt[:], pym[:], AF.Copy)
                if h == 0:
                    ya = work.tile([128, D], bf16, name=f"yacc{bi}",
                                   tag=f"yacc{bi}", bufs=2)
                    nc.vector.tensor_scalar_mul(ya[:], ymt[:],
                                                rsig_col[h][:, tb:tb + 1])
                    out_tiles.append(ya)
                else:
                    yr = work.tile([128, D], bf16, name=f"yar{bi}",
                                   tag=f"yar{bi}", bufs=2)
                    nc.vector.scalar_tensor_tensor(
                        yr[:], ymt[:], rsig_col[h][:, tb:tb + 1],
                        yacc[bi][:], ALU.mult, ALU.add)
                    out_tiles.append(yr)
            return out_tiles

        # =========================================================
        # schedule
        # =========================================================
        for c in range(NCH // 2):
            input_proj_chunk(c)

        load_weights(0)
        # Qpairs[(lyr, P, h)] = transient Q tiles for chunks (2P, 2P+1);
        # zq/rope for a pair is emitted two chunk-groups ahead of its use
        # (pair 0 of each layer inside the previous layer's stream).
        Qpairs = {}
        for h in range(2):
            Qpairs[(0, 0, h)] = zq_rope_pair(0, h, 0)

        def emit_pre(lyr, c):
            """Stage-A work scheduled at the head of group (lyr, c):
            lagged LN chunks, weight prefetch, and led zq/rope pairs."""
            if c < LAG:
                if lyr >= 1:
                    ln_chunk(lyr, c + NCH - LAG)
                else:
                    input_proj_chunk(c + NCH - LAG)
            else:
                if lyr + 1 <= NL:
                    ln_chunk(lyr + 1, c - (NCH - LAG))
            if c == 4 and lyr + 1 < NL:
                load_weights(lyr + 1)
            if c == 5 and lyr == NL - 1:
                for dc in range(DC):
                    nc.sync.dma_start(hw_sb[dc][:], hw_p[dc])
            if c % 2 == 0 and c + 2 < NCH:
                for h in range(2):
                    Qpairs[(lyr, c // 2 + 1, h)] = \
                        zq_rope_pair(lyr, h, c + 2)
            if c == 6 and lyr + 1 < NL:
                for h in range(2):
                    Qpairs[(lyr + 1, 0, h)] = zq_rope_pair(lyr + 1, h, 0)

        steps = [(lyr, c) for lyr in range(NL) for c in range(NCH)]
        for i, (lyr, c) in enumerate(steps):
            emit_pre(lyr, c)
            sy0 = scores_ykv(lyr, 0, c)
            sy1 = scores_ykv(lyr, 1, c)
            stats_rsig(0, c, sy0[1])
            stats_rsig(1, c, sy1[1])
            yacc = attn_tail(lyr, 0, c, Qpairs.pop((lyr, c // 2, 0))
                             if c % 2 else Qpairs[(lyr, c // 2, 0)],
                             sy0[0], sy0[1], None)
            yar = attn_tail(lyr, 1, c, Qpairs.pop((lyr, c // 2, 1))
                             if c % 2 else Qpairs[(lyr, c // 2, 1)],
                             sy1[0], sy1[1], yacc)
            for bi in range(CHUNK // 128):
                nc.sync.dma_start(
                    ar_in[c][bi * 128:(bi + 1) * 128, :], yar[bi][:])
            if lyr == NL - 1 and c >= NCH - 2:
                # split the final AllReduce so the first half's result
                # (and its LN + head tile) overlaps the second half
                for bi in range(CHUNK // 128):
                    rows = slice(bi * 128, (bi + 1) * 128)
                    nc.gpsimd.collective_compute(
                        "AllReduce", ALU.add,
                        replica_groups=[[0, 1, 2, 3], [4, 5, 6, 7]],
                        ins=[ar_in[c][rows, :].opt()],
                        outs=[ar_out[c][rows, :].opt()],
                    )
            else:
                nc.gpsimd.collective_compute(
                    "AllReduce", ALU.add,
                    replica_groups=[[0, 1, 2, 3], [4, 5, 6, 7]],
                    ins=[ar_in[c].opt()], outs=[ar_out[c].opt()],
                )

        # =========================================================
        # head: logitsT [OUT, T] = head_w^T @ xs^T + head_b
        # (all final-layer LN chunks were emitted inside the layer loop)
        # =========================================================
        for c in range(NCH // 2, NCH - 1):
            ln_chunk(NL, c)
        for c in range(NCH):
            if c == 2:
                ln_chunk(NL, NCH - 1)
            c0, c1 = c * CHUNK, (c + 1) * CHUNK
            for ot in range(OUT // 128):
                ph = psum.tile([128, CHUNK], f32, name="ph", tag="sz", bufs=4)
                for dc in range(DC):
                    nc.tensor.matmul(ph[:],
                                     hw_sb[dc][:, ot * 128:(ot + 1) * 128],
                                     half_ap(xsT[dc], c0, c1),
                                     start=(dc == 0), stop=(dc == DC - 1))
                ot_sb = work.tile([128, CHUNK], f32, name="ot_sb", tag="ot_sb",
                                  bufs=2)
                nc.scalar.activation(ot_sb[:], ph[:], AF.Identity,
                                     bias=hb_col[:, ot:ot + 1])
                nc.sync.dma_start(out_p[ot * 128:(ot + 1) * 128, c0:c1],
                                  ot_sb[:])
    nc.compile()
    return nc


def _host_prep(inputs):
    import ml_dtypes

    x = np.asarray(inputs["x"], np.float32)
    w_in = np.asarray(inputs["w_in"], np.float32)
    b_in = np.asarray(inputs["b_in"], np.float32)
    encoder = np.asarray(inputs["encoder"], np.float32)
    encoder_v = np.asarray(inputs["encoder_v"], np.float32)
    decoder = np.asarray(inputs["decoder"], np.float32)
    head_w = np.asarray(inputs["head_w"], np.float32)
    head_b = np.asarray(inputs["head_b"], np.float32)

    perm = np.concatenate([np.arange(0, N, 2), np.arange(1, N, 2)])
    dec3 = decoder.reshape(NH, N, D)
    encp = encoder[:, :, perm].astype(ml_dtypes.bfloat16)
    encvp = encoder_v[:, :, perm].astype(ml_dtypes.bfloat16)
    decp = dec3[:, perm, :].astype(ml_dtypes.bfloat16)
    theta = 2.0 ** 16
    q = np.floor(np.arange(N) / 2.0) * 2.0
    freqs = (1.0 / theta ** (q / N) / (2.0 * math.pi)).astype(np.float32)
    fr = freqs[perm][:NHALF].astype(np.float64)
    ph = (np.arange(T, dtype=np.float64)[None, :] * fr[:, None]) % 1.0
    cosT = np.cos(2 * math.pi * ph).astype(ml_dtypes.bfloat16)
    sinT = np.sin(2 * math.pi * ph).astype(ml_dtypes.bfloat16)
    triu = np.triu(np.ones((128, 128), np.float32), 1)
    maskP = np.concatenate([triu, np.ones((128, 128), np.float32),
                            np.zeros((128, 128), np.float32), triu],
                           axis=1).astype(ml_dtypes.bfloat16)
    ident = np.eye(128, dtype=np.float32).astype(ml_dtypes.bfloat16)
    onesd = np.full((128, 1), 1.0 / D, np.float32).astype(ml_dtypes.bfloat16)
    ones128 = round_fp32r(np.ones((1, 128), np.float32))
    w_inr = round_fp32r(w_in).reshape(DC, 128, D)
    head_wr = head_w.astype(ml_dtypes.bfloat16).reshape(DC, 128, OUT)
    b_in_row = round_fp32r(b_in).reshape(1, D)
    hb_col = np.ascontiguousarray(head_b.reshape(OUT // 128, 128).T)

    in_maps = []
    for c in range(8):
        b = c // 4
        hs = [2 * (c % 4), 2 * (c % 4) + 1]
        in_maps.append({
            "xT": round_fp32r(x[b].T).reshape(DC, 128, T).copy(),
            "w_in": w_inr,
            "b_in_row": b_in_row,
            "enc": encp[hs].reshape(2, DC, 128, N).copy(),
            "encv": encvp[hs].reshape(2, DC, 128, N).copy(),
            "dec": decp[hs].reshape(2, 2, NHALF, D).copy(),
            "cosT": cosT,
            "sinT": sinT,
            "maskP": maskP,
            "ident": ident,
            "onesd": onesd,
            "ones128": ones128,
            "head_w": head_wr,
            "head_b_col": hb_col,
        })
    return in_maps


def kernel(**inputs):
    from concourse.bass_utils import run_bass_kernel_spmd
    global _BUILT, LAST_RESULTS
    if _BUILT is None:
        _BUILT = build()
    in_maps = _host_prep(inputs)
    trace = os.environ.get("KERNEL_TRACE", "0") == "1"
    r = run_bass_kernel_spmd(_BUILT, in_maps, list(range(8)), trace=trace)
    LAST_RESULTS = r
    out = np.empty((B, T, OUT), np.float32)
    for b in range(B):
        out[b] = r.results[4 * b]["logitsT"].T
    return out
